# revision 8
# baseline (speedup 1.0000x reference)
"""ChildSum TreeLSTM on a fixed 8-ary heap tree (N=65536), 8 TRN2 NeuronCores.

Tree facts (verified against the reference tree builder):
  parent(i) = (i-1)//8; children of p are [8p+1, 8p+9); node levels:
    L0 leaves [8192,65536), L1 [1024,8192), L2 [128,1024), L3 [16,128),
    L4 [2,16), L5 {1}, L6 {0}.

Shard scheme (core k of 8) -- SHIFTED so every level's children are exactly
the previous level's own shard (zero cross-core traffic until one tiny
AllGather of L3 results):
  S_leaf: 7168 cols -> [8265+7168k, 15433+7168k)   (>=65536 -> zero pads)
  S_L1:    896 cols -> [1033+896k, 1929+896k)      (core 7 tail 8192-8200 are
           leaves: zeroed pad children reduce the parent pipeline to the leaf
           equations automatically)
  S_L2:    112 cols -> [129+112k, 241+112k)        (core 7 last col = node
           1024, an L1 node whose children 8193-8200 sit in its "L1" shard)
  S_L3:     14 cols -> [16+14k, 30+14k)
  Replicated tail: leaves [8193,8265), nodes 1024-1032, node 128 (computed
  before/while the gather flies), then L4 [2,16), node 1, node 0 after it.

On-device layout is feature-major node-order: h/c/x stored [128 feats, nodes].
Matmul operands bf16; PSUM fp32.  i/o/u gates exploit child-sum linearity:
one contiguous 8-child DVE reduce then a single U matmul per gate.
Output DMAs are issued per-block from the gpsimd queue right after each block
computes, so the drain overlaps compute and the gather window.
"""
import numpy as np
import ml_dtypes

import concourse.bass as bass
import concourse.mybir as mybir
import concourse.tile as tile
from concourse import bacc
from concourse import bass_utils

F32 = mybir.dt.float32
BF16 = mybir.dt.bfloat16
NPBF = ml_dtypes.bfloat16
AF = mybir.ActivationFunctionType
H = 128
N = 65536
NCORE = 8
NLEAF = 7168
NL1 = 896
NL2 = 112
NL3 = 14
SB = 1024           # leaf superblock width
PB = 448            # L1 parent block width
LEAF0 = 8265        # first leaf col (= 8*1033+1)
L1_0 = 1033
L2_0 = 129
L3_0 = 16
# xint column offsets (within the xint block)
XI_L1 = 0
XI_L2 = 896
XI_L3 = 1008
XI_T0 = 1022        # nodes [0,16)
XI_T128 = 1038      # node 128
XI_T1024 = 1039     # nodes [1024,1033)
XI_TLEAF = 1048     # leaves [8193,8265)
XI_W = 1120
# output column offsets
OC_LEAF = 0
OC_L1 = NLEAF                    # 7168
OC_L2 = OC_L1 + NL1              # 8064
OC_L3 = OC_L2 + NL2              # 8176
OC_T0 = OC_L3 + NL3              # 8190: nodes 0..128 (129 cols)
OC_T1024 = OC_T0 + 129           # 8319: nodes 1024..1032 (9)
OC_TLEAF = OC_T1024 + 9          # 8328: leaves 8193..8264 (72)
NOUT = OC_TLEAF + 72             # 8400


def _leaf_gates(nc, P, xa, xb, wc0, wc1, bias, width, outH, outC, mask=None):
    """Dense-only i/o/u gates -> h,c for `width` columns.  outH/outC bf16."""
    def dense(g):
        p = P["psl"].tile([H, width], F32, tag="psl")
        for h0 in range(0, width, 512):
            w = min(512, width - h0)
            nc.tensor.matmul(p[:, h0:h0 + w], wc0[:, g * 128:(g + 1) * 128],
                             xa[:, h0:h0 + w], start=True, stop=False)
            nc.tensor.matmul(p[:, h0:h0 + w], wc1[:, g * 128:(g + 1) * 128],
                             xb[:, h0:h0 + w], start=False, stop=True)
        return p

    ps_i = dense(0)
    ps_u = dense(2)
    si = P["gt"].tile([H, width], BF16, tag="si")
    nc.scalar.activation(si, ps_i, AF.Sigmoid, bias=bias[:, 0:1])
    tu = P["gt"].tile([H, width], BF16, tag="tu")
    nc.scalar.activation(tu, ps_u, AF.Tanh, bias=bias[:, 2:3])
    if mask is not None:
        nc.vector.tensor_mul(si, si, mask)
    nc.vector.tensor_mul(outC, si, tu)
    ps_o = dense(1)
    so = P["gt"].tile([H, width], BF16, tag="so")
    nc.scalar.activation(so, ps_o, AF.Sigmoid, bias=bias[:, 1:2])
    tcx = P["gt"].tile([H, width], BF16, tag="tc")
    nc.scalar.activation(tcx, outC, AF.Tanh)
    nc.vector.tensor_mul(outH, so, tcx)


def _level(nc, P, xint0, xint1, wc0, wc1, u_iou, u_f, bias,
           xoff, npar, chH, chC, choff, outH, outC, oh, pb=PB, red=None):
    """One recurrence level, node-order children: children of local parent j at
    chH/chC cols [choff+8j, choff+8j+8).  All h/c bf16.
    i/o/u: h-sum of 8 children via one grouped reduce, then one U matmul per
    gate.  `red` picks the engine for the 8-child reduces (gpsimd only supports
    partition-axis reduces, so this stays on DVE)."""
    if red is None:
        red = nc.vector
    for pb0 in range(0, npar, pb):
        pw = min(pb, npar - pb0)
        ch_lo = choff + 8 * pb0
        hsum = P["pt"].tile([H, pb], F32, tag="hsum")
        red.tensor_reduce(hsum[:, 0:pw],
                          chH[:, ch_lo:ch_lo + 8 * pw].rearrange("p (n e) -> p n e", e=8),
                          axis=mybir.AxisListType.X, op=mybir.AluOpType.add)
        hsb = P["pt"].tile([H, pb], BF16, tag="hsb")
        nc.vector.tensor_copy(hsb[:, 0:pw], hsum[:, 0:pw])
        sg = {}
        for g, nm in ((0, "i"), (2, "u"), (1, "o")):
            p = P["psa"].tile([H, pw], F32, tag="psa")
            nc.tensor.matmul(p, wc0[:, g * 128:(g + 1) * 128],
                             xint0[:, xoff + pb0:xoff + pb0 + pw], start=True, stop=False)
            nc.tensor.matmul(p, wc1[:, g * 128:(g + 1) * 128],
                             xint1[:, xoff + pb0:xoff + pb0 + pw], start=False, stop=False)
            nc.tensor.matmul(p, u_iou[:, g * 128:(g + 1) * 128], hsb[:, 0:pw],
                             start=False, stop=True)
            s = P["pt"].tile([H, pw], BF16, tag=f"s{nm}")
            nc.scalar.activation(s, p, AF.Tanh if g == 2 else AF.Sigmoid,
                                 bias=bias[:, g:g + 1])
            sg[nm] = s
        # per-child forget gates; fc grouped-sum
        fcs = P["pt"].tile([H, pw], F32, tag="fcs")
        for cb0 in range(0, 8 * pw, 512):
            cw = min(512, 8 * pw - cb0)
            npb = cw // 8
            pf = P["psf"].tile([H, cw], F32, tag="psf")
            xp0 = xint0[:, xoff + pb0 + cb0 // 8:xoff + pb0 + cb0 // 8 + npb]
            xp1 = xint1[:, xoff + pb0 + cb0 // 8:xoff + pb0 + cb0 // 8 + npb]
            nc.tensor.matmul(pf, wc0[:, 384:512],
                             xp0.unsqueeze(2).broadcast_to([H, npb, 8]), start=True, stop=False)
            nc.tensor.matmul(pf, wc1[:, 384:512],
                             xp1.unsqueeze(2).broadcast_to([H, npb, 8]), start=False, stop=False)
            nc.tensor.matmul(pf, u_f, chH[:, ch_lo + cb0:ch_lo + cb0 + cw],
                             start=False, stop=True)
            ft = P["fp"].tile([H, 512], BF16, tag="ft")
            nc.scalar.activation(ft[:, 0:cw], pf, AF.Sigmoid, bias=bias[:, 3:4])
            fct = P["fp"].tile([H, 512], BF16, tag="fct")
            nc.vector.tensor_mul(fct[:, 0:cw], ft[:, 0:cw],
                                 chC[:, ch_lo + cb0:ch_lo + cb0 + cw])
            red.tensor_reduce(fcs[:, cb0 // 8:cb0 // 8 + npb],
                              fct[:, 0:cw].rearrange("p (n e) -> p n e", e=8),
                              axis=mybir.AxisListType.X, op=mybir.AluOpType.add)
        # combine
        ct = P["pt"].tile([H, pw], BF16, tag="ct")
        nc.vector.tensor_mul(ct, sg["i"], sg["u"])
        cs = outC[:, oh + pb0:oh + pb0 + pw]
        nc.vector.tensor_add(cs, ct, fcs)
        tcx = P["pt"].tile([H, pw], BF16, tag="tcx")
        nc.scalar.activation(tcx, cs, AF.Tanh)
        hs = outH[:, oh + pb0:oh + pb0 + pw]
        nc.vector.tensor_mul(hs, sg["o"], tcx)


def build():
    nc = bacc.Bacc("TRN2", target_bir_lowering=False, debug=False, num_devices=NCORE)
    xT = nc.dram_tensor("xT", [256, NLEAF + XI_W], BF16, kind="ExternalInput")
    wcat = nc.dram_tensor("wcat", [256, 512], BF16, kind="ExternalInput")
    uiou = nc.dram_tensor("uiou", [H, 384], BF16, kind="ExternalInput")
    uf = nc.dram_tensor("uf", [H, H], BF16, kind="ExternalInput")
    bias_d = nc.dram_tensor("bias", [H, 4], F32, kind="ExternalInput")
    mask_d = nc.dram_tensor("mask", [H, SB], BF16, kind="ExternalInput")
    h_out = nc.dram_tensor("h_out", [H, NOUT], BF16, kind="ExternalOutput")
    c_out = nc.dram_tensor("c_out", [H, NOUT], BF16, kind="ExternalOutput")

    with tile.TileContext(nc) as tc:
        with (
            tc.tile_pool(name="const", bufs=1) as const,
            tc.tile_pool(name="big", bufs=1) as big,
            tc.tile_pool(name="gt", bufs=4) as gt,
            tc.tile_pool(name="pt", bufs=4) as pt,
            tc.tile_pool(name="fp", bufs=4) as fp,
            tc.tile_pool(name="psl", bufs=2, space="PSUM") as psl,
            tc.tile_pool(name="psa", bufs=2, space="PSUM") as psa,
            tc.tile_pool(name="psf", bufs=2, space="PSUM") as psf,
            tc.tile_pool(name="dram", bufs=1, space="DRAM") as dram,
        ):
            P = {"psl": psl, "psa": psa, "psf": psf, "gt": gt, "pt": pt, "fp": fp}

            # ---- early dummy collective: absorbs cross-core launch skew and
            # warms the ncfw control plane while leaves compute ----
            agid = dram.tile([2, H, 1], BF16, tag="agid")
            agod = dram.tile([NCORE, 2, H, 1], BF16, tag="agod")

            # warm up all DMA rings with tiny transfers, and feed the dummy
            # collective from a memset tile so it fires immediately
            scratch = const.tile([H, 16], BF16, tag="scratch")
            nc.vector.memset(scratch, 0.0)
            wdum = dram.tile([16, H, 1], BF16, tag="wdum")
            for i in range(16):
                nc.sync.dma_start(wdum[i], scratch[:, i:i + 1])
            nc.sync.dma_start(agid[0], scratch[:, 0:1])
            nc.sync.dma_start(agid[1], scratch[:, 1:2])
            nc.gpsimd.collective_compute(
                "AllGather", mybir.AluOpType.bypass,
                replica_groups=[list(range(NCORE))],
                ins=[agid.opt()], outs=[agod.opt()],
            )

            wc0 = const.tile([H, 512], BF16, tag="wc0")
            wc1 = const.tile([H, 512], BF16, tag="wc1")
            nc.sync.dma_start(wc0, wcat.ap()[0:128, :])
            nc.sync.dma_start(wc1, wcat.ap()[128:256, :])
            bias = const.tile([H, 4], F32, tag="bias")
            nc.sync.dma_start(bias, bias_d.ap())

            # persistent x buffers: leaf block + interior block, both halves
            xla = const.tile([H, NLEAF], BF16, tag="xla")
            xlb = const.tile([H, NLEAF], BF16, tag="xlb")
            xint0 = const.tile([H, XI_W], BF16, tag="xint0")
            xint1 = const.tile([H, XI_W], BF16, tag="xint1")
            # leaf x in need-order so early superblocks clear the cold window
            for lo, hi in ((0, SB), (SB, 2 * SB), (2 * SB, 3 * SB)):
                nc.sync.dma_start(xla[:, lo:hi], xT.ap()[0:128, lo:hi])
                nc.sync.dma_start(xlb[:, lo:hi], xT.ap()[128:256, lo:hi])
            u_iou = const.tile([H, 384], BF16, tag="uiou")
            nc.sync.dma_start(u_iou, uiou.ap())
            u_f = const.tile([H, H], BF16, tag="uf")
            nc.sync.dma_start(u_f, uf.ap())
            mask = const.tile([H, SB], BF16, tag="mask")
            nc.sync.dma_start(mask, mask_d.ap())
            nc.sync.dma_start(xint0, xT.ap()[0:128, NLEAF:NLEAF + XI_W])
            nc.sync.dma_start(xint1, xT.ap()[128:256, NLEAF:NLEAF + XI_W])
            for lo, hi in ((3 * SB, 5 * SB), (5 * SB, NLEAF)):
                nc.sync.dma_start(xla[:, lo:hi], xT.ap()[0:128, lo:hi])
                nc.sync.dma_start(xlb[:, lo:hi], xT.ap()[128:256, lo:hi])

            leafH = big.tile([H, NLEAF], BF16, tag="leafH")
            leafC = big.tile([H, NLEAF], BF16, tag="leafC")
            hL1 = big.tile([H, NL1], BF16, tag="hL1")
            cL1 = big.tile([H, NL1], BF16, tag="cL1")
            hL23 = big.tile([H, NL2 + NL3], BF16, tag="hL23")
            cL23 = big.tile([H, NL2 + NL3], BF16, tag="cL23")
            hL2, cL2 = hL23[:, 0:NL2], cL23[:, 0:NL2]
            hL3, cL3 = hL23[:, NL2:NL2 + NL3], cL23[:, NL2:NL2 + NL3]
            # tail state: cols 0..127 = nodes 0..127 (L3 gathered into 16..127),
            # col 128 = node 128
            hS = big.tile([H, 129], BF16, tag="hS")
            cS = big.tile([H, 129], BF16, tag="cS")
            htl = big.tile([H, 72], BF16, tag="htl")
            ctl = big.tile([H, 72], BF16, tag="ctl")
            h1k = big.tile([H, 9], BF16, tag="h1k")
            c1k = big.tile([H, 9], BF16, tag="c1k")

            def leaf_sb(sb):
                lo = sb * SB
                _leaf_gates(nc, P, xla[:, lo:lo + SB], xlb[:, lo:lo + SB],
                            wc0, wc1, bias, SB,
                            leafH[:, lo:lo + SB], leafC[:, lo:lo + SB],
                            mask=mask if sb == NLEAF // SB - 1 else None)

            def l1_pb(i):
                _level(nc, P, xint0, xint1, wc0, wc1, u_iou, u_f, bias,
                       XI_L1 + i * PB, PB, leafH, leafC, 8 * i * PB, hL1, cL1, i * PB)

            # ---- main sharded pipeline: leaves -> L1 -> L2 -> L3 ----
            for sb in range(4):
                leaf_sb(sb)
            l1_pb(0)
            nc.sync.dma_start(h_out.ap()[:, 0:4 * SB], leafH[:, 0:4 * SB])
            nc.sync.dma_start(c_out.ap()[:, 0:4 * SB], leafC[:, 0:4 * SB])
            for sb in range(4, NLEAF // SB):
                leaf_sb(sb)
            l1_pb(1)
            nc.sync.dma_start(h_out.ap()[:, 4 * SB:NLEAF], leafH[:, 4 * SB:NLEAF])
            nc.sync.dma_start(c_out.ap()[:, 4 * SB:NLEAF], leafC[:, 4 * SB:NLEAF])
            nc.sync.dma_start(h_out.ap()[:, OC_L1:OC_L1 + NL1], hL1)
            nc.sync.dma_start(c_out.ap()[:, OC_L1:OC_L1 + NL1], cL1)
            _level(nc, P, xint0, xint1, wc0, wc1, u_iou, u_f, bias,
                   XI_L2, NL2, hL1, cL1, 0, hL2, cL2, 0)
            _level(nc, P, xint0, xint1, wc0, wc1, u_iou, u_f, bias,
                   XI_L3, NL3, hL2, cL2, 0, hL3, cL3, 0)

            # ---- single tiny AllGather of L3 h/c ----
            agi = dram.tile([2, H, NL3], BF16, tag="agi")
            ago = dram.tile([NCORE, 2, H, NL3], BF16, tag="ago")
            nc.sync.dma_start(agi[0], hL3)
            nc.sync.dma_start(agi[1], cL3)
            nc.gpsimd.collective_compute(
                "AllGather", mybir.AluOpType.bypass,
                replica_groups=[list(range(NCORE))],
                ins=[agi.opt()], outs=[ago.opt()],
            )
            nc.sync.dma_start(h_out.ap()[:, OC_L2:OC_L2 + NL2 + NL3], hL23)
            nc.sync.dma_start(c_out.ap()[:, OC_L2:OC_L2 + NL2 + NL3], cL23)

            # ---- replicated tail pieces that don't need the gather ----
            # leaves 8193..8264
            _leaf_gates(nc, P, xint0[:, XI_TLEAF:XI_TLEAF + 72],
                        xint1[:, XI_TLEAF:XI_TLEAF + 72],
                        wc0, wc1, bias, 72, htl, ctl)
            # nodes 1024..1032 (children = htl cols [8j, 8j+8))
            _level(nc, P, xint0, xint1, wc0, wc1, u_iou, u_f, bias,
                   XI_T1024, 9, htl, ctl, 0, h1k, c1k, 0)
            # node 128 (children = nodes 1025..1032 = h1k cols [1,9))
            _level(nc, P, xint0, xint1, wc0, wc1, u_iou, u_f, bias,
                   XI_T128, 1, h1k, c1k, 1, hS[:, 128:129], cS[:, 128:129], 0)
            nc.sync.dma_start(h_out.ap()[:, OC_TLEAF:OC_TLEAF + 72], htl)
            nc.sync.dma_start(c_out.ap()[:, OC_TLEAF:OC_TLEAF + 72], ctl)
            nc.sync.dma_start(h_out.ap()[:, OC_T1024:OC_T1024 + 9], h1k)
            nc.sync.dma_start(c_out.ap()[:, OC_T1024:OC_T1024 + 9], c1k)

            # ---- land the gather into the tail state ----
            hSv = hS[:, 16:128].rearrange("p (b c) -> p b c", b=NCORE)
            cSv = cS[:, 16:128].rearrange("p (b c) -> p b c", b=NCORE)
            nc.sync.dma_start(hSv, ago[:, 0].transpose([1, 0, 2]))
            nc.sync.dma_start(cSv, ago[:, 1].transpose([1, 0, 2]))

            # ---- tail levels L4, L5, L6 ----
            for xo, np_, choff, olo in ((XI_T0 + 2, 14, 17, 2),
                                        (XI_T0 + 1, 1, 9, 1),
                                        (XI_T0, 1, 1, 0)):
                _level(nc, P, xint0, xint1, wc0, wc1, u_iou, u_f, bias,
                       xo, np_, hS, cS, choff,
                       hS[:, olo:olo + np_], cS[:, olo:olo + np_], 0)
            nc.sync.dma_start(h_out.ap()[:, OC_T0:OC_T0 + 129], hS)
            nc.sync.dma_start(c_out.ap()[:, OC_T0:OC_T0 + 129], cS)
    nc.compile()
    return nc


_NC_CACHE = None


def _get_program():
    global _NC_CACHE
    if _NC_CACHE is None:
        _NC_CACHE = build()
    return _NC_CACHE


def _host_prep(x, W_iou, U_iou, b_iou, W_f, U_f, b_f):
    x = np.asarray(x, np.float32)
    xTg = np.ascontiguousarray(x.T.astype(NPBF))  # [256, 65536] bf16
    wcat = np.ascontiguousarray(
        np.concatenate([np.asarray(W_iou, np.float32).T,
                        np.asarray(W_f, np.float32).T], axis=1).astype(NPBF))
    uiou_ = np.ascontiguousarray(np.asarray(U_iou, np.float32).astype(NPBF))
    uf_ = np.ascontiguousarray(np.asarray(U_f, np.float32).astype(NPBF))
    b_iou = np.asarray(b_iou, np.float32)[0]
    b_f = np.asarray(b_f, np.float32)[0]
    bias = np.ascontiguousarray(
        np.stack([b_iou[0:128], b_iou[128:256], b_iou[256:384], b_f], axis=1))

    in_maps = []
    for k in range(NCORE):
        xk = np.zeros((256, NLEAF + XI_W), NPBF)
        lo = LEAF0 + NLEAF * k
        hi = min(lo + NLEAF, N)
        nreal = hi - lo
        xk[:, 0:nreal] = xTg[:, lo:hi]
        xi = NLEAF
        xk[:, xi + XI_L1:xi + XI_L1 + NL1] = xTg[:, L1_0 + NL1 * k:L1_0 + NL1 * (k + 1)]
        xk[:, xi + XI_L2:xi + XI_L2 + NL2] = xTg[:, L2_0 + NL2 * k:L2_0 + NL2 * (k + 1)]
        xk[:, xi + XI_L3:xi + XI_L3 + NL3] = xTg[:, L3_0 + NL3 * k:L3_0 + NL3 * (k + 1)]
        xk[:, xi + XI_T0:xi + XI_T0 + 16] = xTg[:, 0:16]
        xk[:, xi + XI_T128] = xTg[:, 128]
        xk[:, xi + XI_T1024:xi + XI_T1024 + 9] = xTg[:, 1024:1033]
        xk[:, xi + XI_TLEAF:xi + XI_TLEAF + 72] = xTg[:, 8193:8265]
        msk = np.ones((H, SB), NPBF)
        if nreal < NLEAF:
            msk[:, SB - (NLEAF - nreal):] = 0.0
        in_maps.append({"xT": xk, "wcat": wcat, "uiou": uiou_, "uf": uf_,
                        "bias": bias, "mask": msk})
    return in_maps


def _assemble(results):
    h = np.empty((N, H), np.float32)
    c = np.empty((N, H), np.float32)
    for k in range(NCORE):
        ho = np.asarray(results[k]["h_out"]).astype(np.float32)
        co = np.asarray(results[k]["c_out"]).astype(np.float32)
        lo = LEAF0 + NLEAF * k
        hi = min(lo + NLEAF, N)
        h[lo:hi] = ho[:, 0:hi - lo].T
        c[lo:hi] = co[:, 0:hi - lo].T
        h[L1_0 + NL1 * k:L1_0 + NL1 * (k + 1)] = ho[:, OC_L1:OC_L1 + NL1].T
        c[L1_0 + NL1 * k:L1_0 + NL1 * (k + 1)] = co[:, OC_L1:OC_L1 + NL1].T
        h[L2_0 + NL2 * k:L2_0 + NL2 * (k + 1)] = ho[:, OC_L2:OC_L2 + NL2].T
        c[L2_0 + NL2 * k:L2_0 + NL2 * (k + 1)] = co[:, OC_L2:OC_L2 + NL2].T
        h[L3_0 + NL3 * k:L3_0 + NL3 * (k + 1)] = ho[:, OC_L3:OC_L3 + NL3].T
        c[L3_0 + NL3 * k:L3_0 + NL3 * (k + 1)] = co[:, OC_L3:OC_L3 + NL3].T
    ho = np.asarray(results[0]["h_out"]).astype(np.float32)
    co = np.asarray(results[0]["c_out"]).astype(np.float32)
    h[0:129] = ho[:, OC_T0:OC_T0 + 129].T
    c[0:129] = co[:, OC_T0:OC_T0 + 129].T
    h[1024:1033] = ho[:, OC_T1024:OC_T1024 + 9].T
    c[1024:1033] = co[:, OC_T1024:OC_T1024 + 9].T
    h[8193:8265] = ho[:, OC_TLEAF:OC_TLEAF + 72].T
    c[8193:8265] = co[:, OC_TLEAF:OC_TLEAF + 72].T
    return h, c


def run(in_maps, **kw):
    nc = _get_program()
    return bass_utils.run_bass_kernel_spmd(nc, in_maps, core_ids=list(range(NCORE)), **kw)


def kernel(x, W_iou, U_iou, b_iou, W_f, U_f, b_f,
           edge_src=None, edge_dst=None, edge_level=None, node_level=None,
           num_levels=None):
    in_maps = _host_prep(x, W_iou, U_iou, b_iou, W_f, U_f, b_f)
    res = run(in_maps)
    return _assemble(res.results)
